# revision 1
# baseline (speedup 1.0000x reference)
"""Trainium2 Bass kernel for unscaled Luong dot-product attention.

Problem: B=16, Tq=Tk=D=1024, fp32.
    scores = Q @ E^T ; weights = softmax(scores, -1) ; out = weights @ E

Sharding: data-parallel over batch — each of the 8 NeuronCores processes
2 batches end-to-end; no cross-core communication.

Per-core pipeline (per batch, per 128-row q-block):
  1. PE-transpose Q and E tiles (fp32) into [D-part, T-free] layout; the
     PSUM->SBUF copies round the operands to float32r (tf32-like) plus a
     float32r residual term (3xTF32 split) so the QK^T matmul runs at the
     full PE rate with ~fp32-grade accuracy.
  2. bmm1: scores[q,k] accumulated over 3 passes x 8 d-chunks in PSUM.
  3. Softmax along the free dim: DVE reduce_max (negated) -> one ACT exp
     with per-partition bias and fused row-sum accumulation -> DVE
     reciprocal. Normalization is folded into the output copy.
  4. PE-transpose the weights block, round to float32r, and run bmm2
     against E kept in natural [k,d] layout (single fp32r pass).
"""

import numpy as np

import concourse.bass as bass
import concourse.tile as tile
from concourse import bacc, mybir
from concourse.masks import make_identity

P = 128
B_PER_CORE = 2
T = 1024  # Tq = Tk
D = 1024
NC_CHUNKS = T // P  # 8 k-chunks / q-blocks
ND_CHUNKS = D // P  # 8 d-chunks
F32 = mybir.dt.float32
F32R = mybir.dt.float32r


def _transpose_block_group(
    nc, trans_psum, ident, src_fn, dst_r, dst_l, n_blocks=4, copy_eng=None
):
    """Transpose `n_blocks` [128,128] fp32 SBUF blocks through one PSUM bank,
    then round the packed result into float32r `dst_r` and (optionally) the
    residual into float32r `dst_l` (3xTF32 split). src_fn(j) -> source AP.
    copy_eng picks the PSUM->SBUF copy engine (callers alternate ACT/DVE so
    neither engine serializes the transpose chain); residual is DVE-only."""
    tp = trans_psum.tile([P, n_blocks * P], F32)
    for j in range(n_blocks):
        nc.tensor.transpose(tp[:, j * P : (j + 1) * P], src_fn(j), ident)
    if copy_eng is None:
        copy_eng = nc.scalar
    if copy_eng is nc.scalar:
        nc.scalar.copy(dst_r, tp[:])
    else:
        nc.vector.tensor_copy(dst_r, tp[:])
    if dst_l is not None:
        nc.vector.tensor_tensor(dst_l, tp[:], dst_r, mybir.AluOpType.subtract)


def build_nc(
    reps: int = 1,
    npass: int = 3,
    dma_only: bool = False,
    e_hoist: bool = False,
    no_softmax: bool = False,
    e_only: bool = False,
    e_sync_dma: bool = False,
    ldw_min: bool = False,
    deep_bufs: bool = False,
    w_split: bool = True,
    qw_half: bool = False,
    ctx2: bool = False,
    ldw2: bool = False,
):
    nc = bacc.Bacc("TRN2", target_bir_lowering=False, debug=False)
    q_dram = nc.dram_tensor("q", [B_PER_CORE, T, D], F32, kind="ExternalInput").ap()
    e_dram = nc.dram_tensor("e", [B_PER_CORE, T, D], F32, kind="ExternalInput").ap()
    o_dram = nc.dram_tensor("o", [B_PER_CORE, T, D], F32, kind="ExternalOutput").ap()

    with tile.TileContext(nc) as tc:
        with (
            tc.tile_pool(name="const", bufs=1) as const_pool,
            tc.tile_pool(name="e_nat", bufs=3) as e_nat_pool,
            tc.tile_pool(name="e_r", bufs=1) as e_r_pool,
            tc.tile_pool(name="etr", bufs=1) as etr_pool,
            tc.tile_pool(name="etl", bufs=1) as etl_pool,
            tc.tile_pool(name="qstage", bufs=3 if deep_bufs else 2) as q_pool,
            tc.tile_pool(name="qt", bufs=3 if deep_bufs else 2) as qt_pool,
            tc.tile_pool(name="w", bufs=3 if deep_bufs else 2) as w_pool,
            tc.tile_pool(name="wt", bufs=3 if deep_bufs else 2) as wt_pool,
            tc.tile_pool(name="ctx", bufs=3 if deep_bufs else 2) as ctx_pool,
            tc.tile_pool(name="stat", bufs=4) as stat_pool,
            tc.tile_pool(name="sc_ps", bufs=2, space="PSUM") as sc_psum,
            tc.tile_pool(
                name="ctx_ps", bufs=2 if ctx2 else 1, space="PSUM"
            ) as ctx_psum,
            tc.tile_pool(
                name="tr_ps", bufs=2 if ctx2 else 3, space="PSUM"
            ) as trans_psum,
        ):
            ident = const_pool.tile([P, P], F32)
            make_identity(nc, ident[:])

            ngr = 2 if qw_half else 1  # half-tiles per transposed operand
            nper = ND_CHUNKS // ngr  # d/k-chunks per half-tile

            def qsel(tiles, c):
                return tiles[c // nper][:, c % nper, :]

            def emit_e_setup(b):
                # ---- E setup, pipelined per 128-row chunk ----
                # Small staging tiles (bufs=3) let chunk k+1's DMA overlap
                # chunk k's transposes, and let the next batch's E DMAs start
                # during this batch's compute. gpsimd (SWDGE) queue keeps them
                # out of the sync queue behind the output DMAs.
                e_r = e_r_pool.tile([P, NC_CHUNKS, D], F32R, name="e_r")
                etr = etr_pool.tile([P, ND_CHUNKS, T], F32R, name="etr")
                etl = (
                    etl_pool.tile([P, ND_CHUNKS, T], F32R, tag="etl", name="etl")
                    if npass >= 3
                    else None
                )
                dma_eng = nc.sync if e_sync_dma else nc.gpsimd
                for kc in range(NC_CHUNKS):
                    e_stage = e_nat_pool.tile([P, D], F32, name="e_stage")
                    dma_eng.dma_start(
                        e_stage[:], e_dram[b, kc * P : (kc + 1) * P, :]
                    )
                    nc.vector.tensor_copy(e_r[:, kc, :], e_stage[:])
                    # transpose the chunk's 8 [128,128] blocks -> column kc of
                    # each etr[:, dc, :]; pack 4 d-blocks per PSUM bank
                    for g in range(ND_CHUNKS // 4):
                        _transpose_block_group(
                            nc,
                            trans_psum,
                            ident[:],
                            lambda j, kc=kc, g=g: e_stage[
                                :, (g * 4 + j) * P : (g * 4 + j + 1) * P
                            ],
                            etr[:, g * 4 : (g + 1) * 4, kc * P : (kc + 1) * P],
                            etl[:, g * 4 : (g + 1) * 4, kc * P : (kc + 1) * P]
                            if etl is not None
                            else None,
                            copy_eng=nc.scalar if (kc * 2 + g) % 2 == 0 else nc.vector,
                        )
                return e_r, etr, etl

            e_cache = None
            for b in [b for _ in range(reps) for b in range(B_PER_CORE)]:
                if dma_only:
                    for kc in range(NC_CHUNKS):
                        e_stage = e_nat_pool.tile([P, D], F32, name="e_stage")
                        nc.gpsimd.dma_start(
                            e_stage[:], e_dram[b, kc * P : (kc + 1) * P, :]
                        )
                    for qb in range(NC_CHUNKS):
                        qstage = q_pool.tile([P, D], F32, name="qstage")
                        nc.sync.dma_start(
                            qstage[:], q_dram[b, qb * P : (qb + 1) * P, :]
                        )
                        ctx_sb = ctx_pool.tile([P, D], F32, name="ctx_sb")
                        nc.vector.tensor_copy(ctx_sb[:], qstage[:])
                        nc.sync.dma_start(
                            o_dram[b, qb * P : (qb + 1) * P, :], ctx_sb[:]
                        )
                    continue
                if e_cache is None or not e_hoist:
                    e_cache = emit_e_setup(b)
                e_r, etr, etl = e_cache
                if e_only:
                    # ablation: skip all q-block work; touch etr so it isn't dead
                    ctx_sb = ctx_pool.tile([P, D], F32, name="ctx_sb")
                    nc.vector.tensor_copy(ctx_sb[:], etr[:, 0, :])
                    nc.sync.dma_start(o_dram[b, 0:P, :], ctx_sb[:])
                    continue

                def emit_front(qb, b=b, etr=etr, etl=etl):
                    """Stage Q block qb, transpose+split it, run bmm1.
                    Returns the scores PSUM tile."""
                    qstage = q_pool.tile([P, D], F32, name="qstage")
                    nc.sync.dma_start(qstage[:], q_dram[b, qb * P : (qb + 1) * P, :])
                    qtr_t = [
                        qt_pool.tile(
                            [P, ND_CHUNKS // ngr, P], F32R,
                            tag=f"qtr{h}", name=f"qtr{h}",
                        )
                        for h in range(ngr)
                    ]
                    qtl_t = [
                        qt_pool.tile(
                            [P, ND_CHUNKS // ngr, P], F32R,
                            tag=f"qtl{h}", name=f"qtl{h}",
                        )
                        for h in range(ngr)
                    ] if npass >= 2 else None
                    for g in range(ND_CHUNKS // 4):
                        o = (g * 4) % nper
                        _transpose_block_group(
                            nc,
                            trans_psum,
                            ident[:],
                            lambda j, g=g: qstage[
                                :, (g * 4 + j) * P : (g * 4 + j + 1) * P
                            ],
                            qtr_t[(g * 4) // nper][:, o : o + 4, :],
                            qtl_t[(g * 4) // nper][:, o : o + 4, :]
                            if qtl_t is not None
                            else None,
                            copy_eng=nc.scalar if g % 2 == 0 else nc.vector,
                        )

                    # bmm1: bank-contiguous bursts (kh outer), npass x 8 k-chunks
                    sc_ps = sc_psum.tile([P, T], F32, name="sc_ps")
                    pairs = [(qtr_t, etr), (qtl_t, etr), (qtr_t, etl)][:npass]
                    n_acc = len(pairs) * ND_CHUNKS
                    if ldw_min:
                        # group MMs by stationary operand: 1 LDW per 4 MMs
                        groups = {}
                        for lhs, rhs in pairs:
                            groups.setdefault(id(lhs), (lhs, []))[1].append(rhs)
                        seq = []  # (lhs, rhs, kh)
                        for dc in range(ND_CHUNKS):
                            for lhs, rhss in groups.values():
                                for rhs in rhss:
                                    for kh in range(2):
                                        seq.append((lhs, rhs, dc, kh))
                        started = set()
                        for i, (lhs, rhs, dc, kh) in enumerate(seq):
                            nc.tensor.matmul(
                                sc_ps[:, kh * 512 : (kh + 1) * 512],
                                qsel(lhs, dc),
                                rhs[:, dc, kh * 512 : (kh + 1) * 512],
                                start=(kh not in started),
                                stop=(i >= len(seq) - 2),
                            )
                            started.add(kh)
                    else:
                        for kh in range(2):
                            if ldw2 and npass == 3:
                                # group the two qtr-consuming passes per
                                # d-chunk: one weight load serves two MMs,
                                # same PSUM bank throughout the half
                                seq = [
                                    (lhs, rhs, dc)
                                    for dc in range(ND_CHUNKS)
                                    for lhs, rhs in ((qtr_t, etr), (qtr_t, etl))
                                ] + [
                                    (qtl_t, etr, dc) for dc in range(ND_CHUNKS)
                                ]
                            else:
                                seq = [
                                    (lhs, rhs, dc)
                                    for lhs, rhs in pairs
                                    for dc in range(ND_CHUNKS)
                                ]
                            for i, (lhs, rhs, dc) in enumerate(seq):
                                nc.tensor.matmul(
                                    sc_ps[:, kh * 512 : (kh + 1) * 512],
                                    qsel(lhs, dc),
                                    rhs[:, dc, kh * 512 : (kh + 1) * 512],
                                    start=(i == 0),
                                    stop=(i == len(seq) - 1),
                                )
                    return sc_ps

                def emit_back(qb, sc_ps, b=b, e_r=e_r):
                    """Softmax block qb's scores, transpose W, bmm2, store."""
                    recip = stat_pool.tile([P, 1], F32, tag="recip", name="recip")
                    if w_split:
                        # two half-tiles: each half's W transposes start as
                        # soon as its own exp half is done
                        w_halves = [
                            w_pool.tile([P, T // 2], F32, tag=f"w{h}", name=f"w{h}")
                            for h in range(2)
                        ]
                    else:
                        w_sb = w_pool.tile([P, T], F32, name="w_sb")
                        w_halves = [w_sb[:, 0:512], w_sb[:, 512:1024]]
                    if no_softmax:
                        nc.scalar.copy(w_halves[0][:], sc_ps[:, 0:512])
                        nc.scalar.copy(w_halves[1][:], sc_ps[:, 512:1024])
                        nc.vector.memset(recip[:], 1.0)
                    else:
                        negmax = stat_pool.tile(
                            [P, 1], F32, tag="negmax", name="negmax"
                        )
                        nc.vector.tensor_reduce(
                            out=negmax[:],
                            in_=sc_ps[:],
                            op=mybir.AluOpType.max,
                            axis=mybir.AxisListType.X,
                            negate=True,
                        )
                        if w_split:
                            ssums = [
                                stat_pool.tile(
                                    [P, 1], F32, tag=f"ssum{h}", name=f"ssum{h}"
                                )
                                for h in range(2)
                            ]
                            for h in range(2):
                                nc.scalar.activation(
                                    w_halves[h][:],
                                    sc_ps[:, h * 512 : (h + 1) * 512],
                                    mybir.ActivationFunctionType.Exp,
                                    bias=negmax[:],
                                    accum_out=ssums[h][:],
                                )
                            ssum = stat_pool.tile(
                                [P, 1], F32, tag="ssum", name="ssum"
                            )
                            nc.vector.tensor_tensor(
                                ssum[:], ssums[0][:], ssums[1][:],
                                mybir.AluOpType.add,
                            )
                        else:
                            ssum = stat_pool.tile(
                                [P, 1], F32, tag="ssum", name="ssum"
                            )
                            nc.scalar.activation(
                                w_sb[:],
                                sc_ps[:],
                                mybir.ActivationFunctionType.Exp,
                                bias=negmax[:],
                                accum_out=ssum[:],
                            )
                        nc.vector.reciprocal(recip[:], ssum[:])

                    wt_t = [
                        wt_pool.tile(
                            [P, NC_CHUNKS // ngr, P], F32R,
                            tag=f"wt{h}", name=f"wt{h}",
                        )
                        for h in range(ngr)
                    ]
                    for g in range(NC_CHUNKS // 4):
                        wo = (g * 4) % nper
                        _transpose_block_group(
                            nc,
                            trans_psum,
                            ident[:],
                            lambda j, g=g: w_halves[g][
                                :, (j * P) : (j + 1) * P
                            ],
                            wt_t[(g * 4) // nper][:, wo : wo + 4, :],
                            None,
                            copy_eng=nc.scalar if g % 2 == 0 else nc.vector,
                        )

                    # bmm2: ctx[q,d] = WT.T @ E, one PSUM bank per d-half
                    ctx_sb = ctx_pool.tile([P, D], F32, name="ctx_sb")
                    for dh in range(2):
                        ctx_ps = ctx_psum.tile([P, 512], F32, name="ctx_ps")
                        for kc in range(NC_CHUNKS):
                            nc.tensor.matmul(
                                ctx_ps[:],
                                qsel(wt_t, kc),
                                e_r[:, kc, dh * 512 : (dh + 1) * 512],
                                start=(kc == 0),
                                stop=(kc == NC_CHUNKS - 1),
                            )
                        nc.vector.tensor_scalar_mul(
                            ctx_sb[:, dh * 512 : (dh + 1) * 512], ctx_ps[:], recip[:]
                        )
                    nc.sync.dma_start(o_dram[b, qb * P : (qb + 1) * P, :], ctx_sb[:])

                # software pipeline: next block's bmm1 hides this block's softmax
                pend = emit_front(0)
                for qb in range(NC_CHUNKS):
                    nxt = emit_front(qb + 1) if qb + 1 < NC_CHUNKS else None
                    emit_back(qb, pend)
                    pend = nxt

    nc.compile()
    return nc


_NC_CACHE = None


def _get_nc():
    global _NC_CACHE
    if _NC_CACHE is None:
        _NC_CACHE = build_nc()
    return _NC_CACHE


def kernel(decoder_hidden: np.ndarray, encoder_outputs: np.ndarray) -> np.ndarray:
    import os

    # The axon client here has no NTFF profiling hook; make sure a stray
    # BASS_TRACE in the environment can't push run_bass_kernel_spmd onto
    # the tracing path.
    os.environ["BASS_NEVER_TRACE"] = "1"
    from concourse import bass_utils

    dh = np.ascontiguousarray(np.asarray(decoder_hidden, dtype=np.float32))
    eo = np.ascontiguousarray(np.asarray(encoder_outputs, dtype=np.float32))
    assert dh.shape == (16, T, D) and eo.shape == (16, T, D)

    nc = _get_nc()
    in_maps = [
        {
            "q": dh[i * B_PER_CORE : (i + 1) * B_PER_CORE],
            "e": eo[i * B_PER_CORE : (i + 1) * B_PER_CORE],
        }
        for i in range(8)
    ]
    res = bass_utils.run_bass_kernel_spmd(nc, in_maps, core_ids=list(range(8)))
    return np.concatenate([r["o"] for r in res.results], axis=0)



# revision 2
# speedup vs baseline: 2.5627x; 2.5627x over previous
"""Trainium2 Bass kernel for unscaled Luong dot-product attention.

Problem: B=16, Tq=Tk=D=1024, fp32.
    scores = Q @ E^T ; weights = softmax(scores, -1) ; out = weights @ E

Sharding: data-parallel over batch — each of the 8 NeuronCores processes
2 batches end-to-end; no cross-core communication.

Numerics: a single fp32r (TF32-like full-rate) pass is used for both
matmuls. Measured on hardware this gives rel_l2 ~8e-4 vs the fp32
reference (gate is 2e-2) — the fp32r multiplier keeps far more mantissa
than a 10-bit TF32 would.

Per-core pipeline (per batch):
  E setup (per 128-row chunk): DMA -> round to f32r (e_r, also bmm2's
  rhs) -> PE-transpose the f32r chunk (1.5 cyc/row) into etr[d, k].
  Double-buffered across batches so batch b+1's E setup hides under
  batch b's tail q-blocks.

  Per 128-row q-block (software-pipelined: block qb+1's front overlaps
  block qb's back):
    front: DMA Q block, PE-transpose fp32 (round to f32r in the
      PSUM->SBUF copy), bmm1 into PSUM with kh-outer order so the
      row-max of the first half starts at the halfway point.
    back: negated row-max halves (DVE) -> exp with per-partition bias
      and fused row-sum (ACT, f32r output halves) -> PE-transpose W
      halves (f32r) -> bmm2 kc-outer so it starts after the first W
      half -> fold 1/rowsum into the PSUM->SBUF output copy -> DMA out.
"""

import numpy as np

import concourse.bass as bass
import concourse.tile as tile
from concourse import bacc, mybir
from concourse.masks import make_identity

P = 128
B_PER_CORE = 2
T = 1024  # Tq = Tk
D = 1024
NC_CHUNKS = T // P  # 8 k-chunks / q-blocks
ND_CHUNKS = D // P  # 8 d-chunks
F32 = mybir.dt.float32
F32R = mybir.dt.float32r


def build_nc(reps: int = 1):
    nc = bacc.Bacc("TRN2", target_bir_lowering=False, debug=False)
    q_dram = nc.dram_tensor("q", [B_PER_CORE, T, D], F32, kind="ExternalInput").ap()
    e_dram = nc.dram_tensor("e", [B_PER_CORE, T, D], F32, kind="ExternalInput").ap()
    o_dram = nc.dram_tensor("o", [B_PER_CORE, T, D], F32, kind="ExternalOutput").ap()

    with tile.TileContext(nc) as tc:
        with (
            tc.tile_pool(name="const", bufs=1) as const_pool,
            tc.tile_pool(name="e_nat", bufs=3) as e_nat_pool,
            tc.tile_pool(name="e_r", bufs=2) as e_r_pool,
            tc.tile_pool(name="etr", bufs=2) as etr_pool,
            tc.tile_pool(name="qstage", bufs=2) as q_pool,
            tc.tile_pool(name="qt", bufs=2) as qt_pool,
            tc.tile_pool(name="w", bufs=2) as w_pool,
            tc.tile_pool(name="wt", bufs=2) as wt_pool,
            tc.tile_pool(name="ctx", bufs=2) as ctx_pool,
            tc.tile_pool(name="stat", bufs=4) as stat_pool,
            tc.tile_pool(name="sc_ps", bufs=2, space="PSUM") as sc_psum,
            tc.tile_pool(name="ctx_ps", bufs=1, space="PSUM") as ctx_psum,
            tc.tile_pool(name="tr_ps", bufs=2, space="PSUM") as trans_psum,
        ):
            ident = const_pool.tile([P, P], F32)
            make_identity(nc, ident[:])
            ident_r = const_pool.tile([P, P], F32R)
            nc.vector.tensor_copy(ident_r[:], ident[:])

            def transpose_group(src_fn, dst, dtype, copy_eng):
                """Transpose 4 [128,128] blocks through one PSUM bank, then
                copy the packed result into `dst` (f32r, rounding if the
                source was fp32). src_fn(j) -> source AP of block j."""
                tp = trans_psum.tile([P, 4 * P], dtype, name="tp")
                for j in range(4):
                    nc.tensor.transpose(
                        tp[:, j * P : (j + 1) * P],
                        src_fn(j),
                        ident[:] if dtype == F32 else ident_r[:],
                    )
                if copy_eng is nc.scalar:
                    nc.scalar.copy(dst, tp[:])
                else:
                    nc.vector.tensor_copy(dst, tp[:])

            def emit_e_setup(b):
                """Per-batch E preparation: e_r (f32r, natural layout, bmm2
                rhs) and etr (f32r, [d-part, dc, k] transposed, bmm1 rhs)."""
                e_r = e_r_pool.tile([P, NC_CHUNKS, D], F32R, name="e_r")
                etr = etr_pool.tile([P, ND_CHUNKS, T], F32R, name="etr")
                for kc in range(NC_CHUNKS):
                    e_stage = e_nat_pool.tile([P, D], F32, name="e_stage")
                    nc.gpsimd.dma_start(
                        e_stage[:], e_dram[b, kc * P : (kc + 1) * P, :]
                    )
                    nc.vector.tensor_copy(e_r[:, kc, :], e_stage[:])
                    for g in range(2):
                        transpose_group(
                            lambda j, kc=kc, g=g: e_r[
                                :, kc, (g * 4 + j) * P : (g * 4 + j + 1) * P
                            ],
                            etr[:, g * 4 : (g + 1) * 4, kc * P : (kc + 1) * P],
                            F32R,
                            copy_eng=nc.scalar if (kc * 2 + g) % 2 == 0 else nc.vector,
                        )
                return e_r, etr

            for b in [b for _ in range(reps) for b in range(B_PER_CORE)]:
                e_r, etr = emit_e_setup(b)

                def emit_front(qb, b=b, etr=etr):
                    """Stage Q block qb, transpose it (fp32, rounded to f32r
                    in the PSUM->SBUF copy), run 1-pass bmm1. Returns the
                    scores PSUM tile."""
                    qstage = q_pool.tile([P, D], F32, name="qstage")
                    nc.sync.dma_start(qstage[:], q_dram[b, qb * P : (qb + 1) * P, :])
                    qtr = qt_pool.tile([P, ND_CHUNKS, P], F32R, name="qtr")
                    for g in range(2):
                        transpose_group(
                            lambda j, g=g: qstage[
                                :, (g * 4 + j) * P : (g * 4 + j + 1) * P
                            ],
                            qtr[:, g * 4 : (g + 1) * 4, :],
                            F32,
                            copy_eng=nc.scalar if g % 2 == 0 else nc.vector,
                        )

                    # bmm1, kh outer: bank kh finishes all 8 d-chunk
                    # accumulations before bank kh+1 starts, so the softmax
                    # row-max of half 0 overlaps half 1's matmuls.
                    sc_ps = sc_psum.tile([P, T], F32, name="sc_ps")
                    for kh in range(2):
                        for dc in range(ND_CHUNKS):
                            nc.tensor.matmul(
                                sc_ps[:, kh * 512 : (kh + 1) * 512],
                                qtr[:, dc, :],
                                etr[:, dc, kh * 512 : (kh + 1) * 512],
                                start=(dc == 0),
                                stop=(dc == ND_CHUNKS - 1),
                            )
                    return sc_ps

                def emit_back(qb, sc_ps, b=b, e_r=e_r):
                    """Softmax block qb's scores, transpose W, bmm2, store."""
                    # negated row-max, per 512-half (reduce h=0 overlaps
                    # bmm1's second half), combined with min (-max).
                    nmaxes = [
                        stat_pool.tile([P, 1], F32, tag=f"nmax{h}", name=f"nmax{h}")
                        for h in range(2)
                    ]
                    for h in range(2):
                        nc.vector.tensor_reduce(
                            out=nmaxes[h][:],
                            in_=sc_ps[:, h * 512 : (h + 1) * 512],
                            op=mybir.AluOpType.max,
                            axis=mybir.AxisListType.X,
                            negate=True,
                        )
                    negmax = stat_pool.tile([P, 1], F32, tag="negmax", name="negmax")
                    nc.vector.tensor_tensor(
                        negmax[:], nmaxes[0][:], nmaxes[1][:], mybir.AluOpType.min
                    )

                    # exp halves (f32r output) with fused row-sum accumulation
                    w_halves = [
                        w_pool.tile([P, T // 2], F32R, tag=f"w{h}", name=f"w{h}")
                        for h in range(2)
                    ]
                    ssums = [
                        stat_pool.tile([P, 1], F32, tag=f"ssum{h}", name=f"ssum{h}")
                        for h in range(2)
                    ]
                    wt = wt_pool.tile([P, NC_CHUNKS, P], F32R, name="wt")
                    for h in range(2):
                        nc.scalar.activation(
                            w_halves[h][:],
                            sc_ps[:, h * 512 : (h + 1) * 512],
                            mybir.ActivationFunctionType.Exp,
                            bias=negmax[:],
                            accum_out=ssums[h][:],
                        )
                        transpose_group(
                            lambda j, h=h: w_halves[h][:, j * P : (j + 1) * P],
                            wt[:, h * 4 : (h + 1) * 4, :],
                            F32R,
                            copy_eng=nc.scalar if h == 0 else nc.vector,
                        )
                    ssum = stat_pool.tile([P, 1], F32, tag="ssum", name="ssum")
                    nc.vector.tensor_tensor(
                        ssum[:], ssums[0][:], ssums[1][:], mybir.AluOpType.add
                    )
                    recip = stat_pool.tile([P, 1], F32, tag="recip", name="recip")
                    nc.vector.reciprocal(recip[:], ssum[:])

                    # bmm2: ctx[q,d] = WT.T @ E. kc outer so matmuls start
                    # once the first W half's transposes land; dh inner
                    # alternates the two PSUM banks of one [P, 1024] tile.
                    ctx_ps = ctx_psum.tile([P, T], F32, name="ctx_ps")
                    for kc in range(NC_CHUNKS):
                        for dh in range(2):
                            nc.tensor.matmul(
                                ctx_ps[:, dh * 512 : (dh + 1) * 512],
                                wt[:, kc, :],
                                e_r[:, kc, dh * 512 : (dh + 1) * 512],
                                start=(kc == 0),
                                stop=(kc == NC_CHUNKS - 1),
                            )
                    ctx_sb = ctx_pool.tile([P, D], F32, name="ctx_sb")
                    nc.vector.tensor_scalar_mul(ctx_sb[:], ctx_ps[:], recip[:])
                    nc.sync.dma_start(o_dram[b, qb * P : (qb + 1) * P, :], ctx_sb[:])

                # software pipeline: next block's bmm1 hides this block's
                # softmax + W transpose + bmm2 tail latency.
                pend = emit_front(0)
                for qb in range(NC_CHUNKS):
                    nxt = emit_front(qb + 1) if qb + 1 < NC_CHUNKS else None
                    emit_back(qb, pend)
                    pend = nxt

    nc.compile()
    return nc


_NC_CACHE = None


def _get_nc():
    global _NC_CACHE
    if _NC_CACHE is None:
        _NC_CACHE = build_nc()
    return _NC_CACHE


def kernel(decoder_hidden: np.ndarray, encoder_outputs: np.ndarray) -> np.ndarray:
    import os

    # The axon client here has no NTFF profiling hook; make sure a stray
    # BASS_TRACE in the environment can't push run_bass_kernel_spmd onto
    # the tracing path.
    os.environ["BASS_NEVER_TRACE"] = "1"
    from concourse import bass_utils

    dh = np.ascontiguousarray(np.asarray(decoder_hidden, dtype=np.float32))
    eo = np.ascontiguousarray(np.asarray(encoder_outputs, dtype=np.float32))
    assert dh.shape == (16, T, D) and eo.shape == (16, T, D)

    nc = _get_nc()
    in_maps = [
        {
            "q": dh[i * B_PER_CORE : (i + 1) * B_PER_CORE],
            "e": eo[i * B_PER_CORE : (i + 1) * B_PER_CORE],
        }
        for i in range(8)
    ]
    res = bass_utils.run_bass_kernel_spmd(nc, in_maps, core_ids=list(range(8)))
    return np.concatenate([r["o"] for r in res.results], axis=0)
